# revision 21
# baseline (speedup 1.0000x reference)
"""Trainium2 Bass kernel for nn_BayesianEncoder (gnn_message_passing).

Strategy (8 NeuronCores, one TRN2 chip):
  - Batch (200) split into two halves: cores 0-3 take rows 0:100, cores 4-7
    rows 100:200.  Within each quad the EDGES of every sparse layer are split
    4 ways by (sorted) destination node, so the segment-sum scatter stays
    local per core.
  - Per layer: node-feature table lives in DRAM feature-major (rows = nodes,
    128 f32 cols = padded batch half).  GPSIMD dma_gather pulls one 512B row
    per edge into SBUF as (128 edge-lanes, chunk, 128).  The segment-sum is a
    PE matmul per 128-edge chunk with a weighted one-hot stationary operand
    (built on DVE from an iota compare), accumulating into a PSUM window of
    32 (lin) / 64 (pool) destination rows; 128-row groups of windows share a
    PSUM tile via tile_position column offsets.
  - BatchNorm batch statistics are pair-wise AllReduced across the two batch
    halves ({c, c+4}); normalize+ReLU run on ACT with per-partition
    scale/bias; the dropout mask (host-precomputed, deterministic jax RNG)
    multiplies on DVE.  The normalized slice is AllGathered within each quad
    to rebuild the full gather table for the next layer.
  - Bayesian weights w = mu + exp(logsig)*eps and the KL term are computed on
    host (they only depend on the weight vectors, not on activations).
"""

import os
import numpy as np

import concourse.bacc as bacc
import concourse.tile as tile
from concourse import mybir
from concourse.bass_utils import run_bass_kernel_spmd

dt = mybir.dt

# ---------------- problem constants (hardcoded from the reference) --------
B = 200
S_LEVELS = [30000, 15000, 7500, 3750, 1875, 938, 469]
LIN_DEG, POOL_DEG = 16, 2
EPS_BN = 1e-5
DROP_P = 0.1


def _build_layers():
    layers = [(0, 0, LIN_DEG * S_LEVELS[0])]
    for k in range(6):
        layers.append((k, k + 1, POOL_DEG * S_LEVELS[k]))
        layers.append((k + 1, k + 1, LIN_DEG * S_LEVELS[k + 1]))
    return layers


# ---------------- sharding / kernel config --------------------------------
N_CORES = 8
BPAD = 128         # padded batch columns in gather tables (512B rows)
M_LIN = 32         # one-hot window width, lin layers
M_POOL = 64        # one-hot window width, pool layers
GROUP = 128        # psum group rows (= partitions)
GCH = 8            # chunks per dma_gather instruction
VDT_NAME = os.environ.get("KERNEL_DTYPE", "fp32")
DMA_SCRATCH = 32768   # SBUF SWDGE desc carveout bytes/partition (2048/queue)
VDT = None  # set below once dt import is live
VNP = None


def _set_vdt(name):
    global VDT, VNP, VDT_NAME
    VDT_NAME = name
    VDT = dt.float16 if name == "fp16" else dt.float32
    VNP = np.float16 if name == "fp16" else np.float32
OHG = 16           # chunks per one-hot build instruction


def _set_config(b, s_levels):
    """(Re)compute derived globals.  Exists so a scaled-down config can be
    injected for fast simulator tests; the defaults match the real problem."""
    global B, S_LEVELS, LAYERS, NNZ, OUT_SIZE, IN_SIZE, NNZ_OFF, OUT_OFF
    global BN_OFF, N_LAYERS, IS_LIN, BH, BW
    B = b
    S_LEVELS = s_levels
    LAYERS = _build_layers()
    NNZ = [l[2] for l in LAYERS]
    OUT_SIZE = [S_LEVELS[l[1]] for l in LAYERS]
    IN_SIZE = [S_LEVELS[l[0]] for l in LAYERS]
    NNZ_OFF = np.cumsum([0] + NNZ).tolist()
    OUT_OFF = np.cumsum([0] + OUT_SIZE).tolist()
    BN_OFF = np.cumsum([0] + S_LEVELS[:6]).tolist()
    N_LAYERS = len(LAYERS)
    IS_LIN = [l[0] == l[1] for l in LAYERS]
    BH = B // 2
    BW = BH


_set_config(B, S_LEVELS)
_set_vdt(VDT_NAME)


# ---------------- host-side preprocessing ----------------------------------
def _balanced_group_cuts(edge_group_counts, n_parts):
    """Split groups into n_parts contiguous runs with ~equal edge counts."""
    csum = np.concatenate([[0], np.cumsum(edge_group_counts)])
    total = csum[-1]
    cuts = [0]
    for q in range(1, n_parts):
        target = total * q / n_parts
        g = int(np.searchsorted(csum, target))
        g = max(cuts[-1], min(g, len(edge_group_counts) - (n_parts - q)))
        cuts.append(g)
    cuts.append(len(edge_group_counts))
    return cuts  # group index boundaries, len n_parts+1


def _wrap_idxs(idx_linear):
    """Linear idx list (multiple of 128) -> (128, n/16) int16 wrapped."""
    n = len(idx_linear)
    a = idx_linear.reshape(n // 16, 16).T.astype(np.int16)
    return np.tile(a, (8, 1))


class LayerPlan:
    pass


def prepare(edges, w_all, bias):
    """Plan all 13 layers.  Returns (plans, per_core_data).

    per_core_data: list of 4 dicts (quad position q) with the per-layer input
    arrays.  Cores q and q+4 share everything except the x/mask inputs.
    """
    plans = []
    # per quad position: dict name -> np.ndarray
    data = [dict() for _ in range(4)]

    # node id -> padded table row, for the CURRENT layer input level
    # layer 0 reads x directly: identity mapping
    row_of_node = np.arange(S_LEVELS[0], dtype=np.int64)
    table_rows = S_LEVELS[0]

    for li, (a, b, nnz) in enumerate(LAYERS):
        p = LayerPlan()
        p.li = li
        p.lin = IS_LIN[li]
        p.M = M_LIN if p.lin else M_POOL
        p.win_per_group = GROUP // p.M
        S_out = OUT_SIZE[li]
        e0, e1 = NNZ_OFF[li], NNZ_OFF[li + 1]
        src = np.asarray(edges[0, e0:e1], dtype=np.int64)
        dst = np.asarray(edges[1, e0:e1], dtype=np.int64)
        w = np.asarray(w_all[e0:e1], dtype=np.float32)

        order = np.argsort(dst, kind="stable")
        ssrc, sdst, sw = src[order], dst[order], w[order]

        n_groups_tot = -(-S_out // GROUP)
        grp_of_edge = sdst // GROUP
        grp_counts = np.bincount(grp_of_edge, minlength=n_groups_tot)
        cuts = _balanced_group_cuts(grp_counts, 4)
        p.cuts_groups = cuts
        p.row_start = [c * GROUP for c in cuts[:4]]  # node id where slice starts
        p.G = max(cuts[q + 1] - cuts[q] for q in range(4))  # groups per core
        p.P = p.G * GROUP  # padded rows per core slice

        edge_bounds = np.searchsorted(grp_of_edge, cuts)

        # --- per quad position: window edge counts on the COMMON local grid ---
        n_windows = p.G * p.win_per_group  # local window slots per core
        per_q = []
        win_counts_q = np.zeros((4, n_windows), np.int64)
        for q in range(4):
            lo, hi = edge_bounds[q], edge_bounds[q + 1]
            qsrc, qdst, qw = ssrc[lo:hi], sdst[lo:hi], sw[lo:hi]
            row0 = p.row_start[q]
            win_of_edge = (qdst - row0) // p.M
            win_counts_q[q] = np.bincount(win_of_edge, minlength=n_windows)
            per_q.append((qsrc, qdst, qw, row0))

        # Each core assigns its windows to schedule slots sorted by edge count
        # (descending).  Rank-matching across cores minimizes the union
        # chunk-count padding; the window->slot permutation only relabels
        # output rows, which all downstream consumers read from data tables.
        perm_q = [np.argsort(-win_counts_q[q], kind="stable") for q in range(4)]
        sorted_counts = np.stack(
            [win_counts_q[q][perm_q[q]] for q in range(4)])  # (4, n_windows)
        nck_slot = np.maximum(1, -(-sorted_counts.max(axis=0) // 128))
        sched = []
        for j in range(n_windows):
            g_loc = j // p.win_per_group
            w_in_g = j % p.win_per_group
            n = int(nck_slot[j])
            for ci in range(n):
                sched.append((g_loc, w_in_g, ci == 0, ci == n - 1))
        p.sched = sched
        p.NCH = len(sched)

        # node_of_row[q][r]: node id held at local row r of core q's slice
        p.node_of_row = []
        for q in range(4):
            row0 = per_q[q][3]
            end_q = min(cuts[q + 1] * GROUP, S_out)
            nor = np.full(p.P, -1, np.int64)
            for j in range(n_windows):
                w = int(perm_q[q][j])
                n0 = row0 + w * p.M
                n1 = min(n0 + p.M, end_q)
                if n1 > n0:
                    nor[j * p.M: j * p.M + (n1 - n0)] = np.arange(n0, n1)
            p.node_of_row.append(nor)

        # build lane arrays per core following the union schedule
        for q in range(4):
            qsrc, qdst, qw, row0 = per_q[q]
            woff = np.concatenate([[0], np.cumsum(win_counts_q[q])])
            src_l = np.zeros((p.NCH, 128), np.int64)
            dstl_l = np.full((p.NCH, 128), -1.0, np.float32)
            w_l = np.zeros((p.NCH, 128), np.float32)
            ci = 0
            for j in range(n_windows):
                w = int(perm_q[q][j])
                cnt = int(win_counts_q[q][w])
                base = int(woff[w])
                nbase = row0 + w * p.M  # node id at window start
                for k in range(int(nck_slot[j])):
                    s = base + k * 128
                    ln = max(0, min(128, cnt - k * 128))
                    if ln > 0:
                        src_l[ci, :ln] = row_of_node[qsrc[s:s + ln]]
                        dstl_l[ci, :ln] = (qdst[s:s + ln] - nbase).astype(np.float32)
                        w_l[ci, :ln] = qw[s:s + ln]
                    ci += 1
            assert ci == p.NCH
            data[q][f"idx_{li}"] = _wrap_idxs(src_l.reshape(-1))
            data[q][f"dstl_{li}"] = np.ascontiguousarray(dstl_l.T)  # (128, nch)
            data[q][f"w_{li}"] = np.ascontiguousarray(w_l.T).astype(VNP)  # (128, nch)

            # bias lanes (128, G): lane (p, g) holds bias of node at row g*128+p
            bvec = bias[OUT_OFF[li]:OUT_OFF[li + 1]]
            nor = p.node_of_row[q]
            bl = np.where(nor >= 0, bvec[np.clip(nor, 0, None)], 0.0)
            data[q][f"biasl_{li}"] = np.ascontiguousarray(
                bl.reshape(p.G, GROUP).T.astype(np.float32))

        # mapping node -> padded table row for the NEXT layer input
        row_of_node = np.zeros(S_out, np.int64)
        for q in range(4):
            nor = p.node_of_row[q]
            valid = nor >= 0
            row_of_node[nor[valid]] = q * p.P + np.nonzero(valid)[0]
        table_rows = 4 * p.P
        p.table_rows = table_rows
        assert table_rows < 32768, f"layer {li}: table rows {table_rows} exceed int16"
        plans.append(p)

    return plans, data


def _norm_consts_inputs(data, plans, gamma, beta):
    """gamma/beta lanes per lin stage, laid out like the bias lanes."""
    for stage in range(6):
        li = 2 * stage
        p = plans[li]
        gv = gamma[BN_OFF[stage]:BN_OFF[stage + 1]]
        bv = beta[BN_OFF[stage]:BN_OFF[stage + 1]]
        for q in range(4):
            nor = p.node_of_row[q]
            safe = np.clip(nor, 0, None)
            gl = np.where(nor >= 0, gv[safe], 1.0)
            bl = np.where(nor >= 0, bv[safe], 0.0)
            data[q][f"gammal_{stage}"] = np.ascontiguousarray(
                gl.reshape(p.G, GROUP).T.astype(np.float32))
            data[q][f"betal_{stage}"] = np.ascontiguousarray(
                bl.reshape(p.G, GROUP).T.astype(np.float32))


def _mask_inputs(data, plans, masks):
    """Dropout masks (already scaled by 1/(1-p)), feature-major slices.

    masks[stage]: (B, S_stage) f32.  Stored per (quad q, batch half h) as
    (G*128, BW); cores q and q+4 differ here.
    """
    out = [dict() for _ in range(8)]
    for stage in range(6):
        li = 2 * stage
        p = plans[li]
        m = masks[stage]
        for q in range(4):
            nor = p.node_of_row[q]
            safe = np.clip(nor, 0, None)
            for h in range(2):
                sl = m[h * BH:(h + 1) * BH, safe].T.copy()
                sl[nor < 0] = 0.0
                out[h * 4 + q][f"mask_{stage}"] = np.ascontiguousarray(
                    sl.astype(VNP))
    return out


# ---------------- program builder ------------------------------------------
def build_program(plans, collectives=True, limit=None, skip_norm=False, skip_accum=False):
    nc = bacc.Bacc("TRN2", target_bir_lowering=False, debug=False,
                   num_devices=N_CORES if collectives else 1,
                   dynamic_dma_scratch_size=DMA_SCRATCH)

    AG_GROUPS = [[0, 1, 2, 3], [4, 5, 6, 7]]
    AR_GROUPS = [[0, 4], [1, 5], [2, 6], [3, 7]]

    # ---- dram I/O ----
    xT = nc.dram_tensor("xT", [S_LEVELS[0], BPAD], VDT,
                        kind="ExternalInput").ap()
    iota_in = nc.dram_tensor("iota", [128, M_POOL], dt.float32,
                             kind="ExternalInput").ap()
    ins = {}
    for li, p in enumerate(plans):
        ins[f"idx_{li}"] = nc.dram_tensor(
            f"idx_{li}", [128, p.NCH * 8], dt.int16, kind="ExternalInput").ap()
        ins[f"dstl_{li}"] = nc.dram_tensor(
            f"dstl_{li}", [128, p.NCH], dt.float32, kind="ExternalInput").ap()
        ins[f"w_{li}"] = nc.dram_tensor(
            f"w_{li}", [128, p.NCH], VDT, kind="ExternalInput").ap()
        ins[f"biasl_{li}"] = nc.dram_tensor(
            f"biasl_{li}", [128, p.G], dt.float32, kind="ExternalInput").ap()
    for stage in range(6):
        p = plans[2 * stage]
        ins[f"gammal_{stage}"] = nc.dram_tensor(
            f"gammal_{stage}", [128, p.G], dt.float32, kind="ExternalInput").ap()
        ins[f"betal_{stage}"] = nc.dram_tensor(
            f"betal_{stage}", [128, p.G], dt.float32, kind="ExternalInput").ap()
        ins[f"mask_{stage}"] = nc.dram_tensor(
            f"mask_{stage}", [p.G * GROUP, BW], VDT,
            kind="ExternalInput").ap()

    if limit is None:
        limit = len(plans)
    pF = plans[limit - 1]
    out_final = nc.dram_tensor("out_final", [pF.G * GROUP, BW], dt.float32,
                               kind="ExternalOutput").ap()

    # internal DRAM: gather tables (allgather outputs) + bounce buffers
    tables = []
    ag_ins = []
    for li, p in enumerate(plans[:-1]):
        tables.append(nc.dram_tensor(
            f"table_{li}", [4 * p.P, BPAD], VDT).ap())
        ag_ins.append(nc.dram_tensor(
            f"agin_{li}", [p.P, BPAD], VDT).ap())
    stats_bounce = []
    lin_layers = [li for li in range(N_LAYERS) if IS_LIN[li]]
    for li in lin_layers:
        p = plans[li]
        stats_bounce.append((
            nc.dram_tensor(f"stin_{li}", [128, 2 * p.G], dt.float32).ap(),
            nc.dram_tensor(f"stout_{li}", [128, 2 * p.G], dt.float32).ap(),
        ))
    stats_of = {li: stats_bounce[i] for i, li in enumerate(lin_layers)}

    with tile.TileContext(nc) as tc:
        with (
            tc.tile_pool(name="vals", bufs=3) as vals_pool,
            tc.tile_pool(name="oh", bufs=4) as oh_pool,
            tc.tile_pool(name="meta", bufs=2) as meta_pool,
            tc.tile_pool(name="h", bufs=64) as h_pool,
            tc.tile_pool(name="stat", bufs=4) as stat_pool,
            tc.tile_pool(name="small", bufs=8) as small_pool,
            tc.tile_pool(name="cst", bufs=1) as cst_pool,
            tc.tile_pool(name="ps", bufs=6, space="PSUM") as ps_pool,
        ):
            iota_t = cst_pool.tile([128, M_POOL], dt.float32)
            nc.sync.dma_start(iota_t[:], iota_in[:])

            for li, p in enumerate(plans[:limit]):
                M = p.M
                table = xT if li == 0 else tables[li - 1][:]

                # --- per-layer metadata loads ---
                idx_t = meta_pool.tile([128, p.NCH * 8], dt.int16, tag="idx")
                nc.sync.dma_start(idx_t[:], ins[f"idx_{li}"][:])
                dstl_t = meta_pool.tile([128, p.NCH], dt.float32, tag="dstl")
                nc.sync.dma_start(dstl_t[:], ins[f"dstl_{li}"][:])
                w_t = meta_pool.tile([128, p.NCH], VDT, tag="w")
                nc.sync.dma_start(w_t[:], ins[f"w_{li}"][:])
                bias_t = small_pool.tile([128, p.G], dt.float32, tag="bias")
                nc.sync.dma_start(bias_t[:], ins[f"biasl_{li}"][:])

                # --- gathers ---
                n_g = -(-p.NCH // GCH)
                vals_tiles = []
                for gi in range(n_g):
                    c0 = gi * GCH
                    ncur = min(GCH, p.NCH - c0)
                    vt = vals_pool.tile([128, GCH, BPAD], VDT)
                    nc.gpsimd.dma_gather(
                        out_ap=vt[:, :ncur, :],
                        in_ap=table,
                        idxs_ap=idx_t[:, c0 * 8:(c0 + ncur) * 8],
                        num_idxs=ncur * 128,
                        num_idxs_reg=ncur * 128,
                        elem_size=BPAD,
                    )
                    vals_tiles.append(vt)

                # --- one-hot builds ---
                n_oh = -(-p.NCH // OHG)
                oh_tiles = []
                for oi in range(n_oh):
                    c0 = oi * OHG
                    ncur = min(OHG, p.NCH - c0)
                    ot = oh_pool.tile([128, OHG, M], VDT)
                    dstl_b = dstl_t[:, c0:c0 + ncur].unsqueeze(2) \
                        .broadcast_to([128, ncur, M])
                    iota_b = iota_t[:, :M].unsqueeze(1) \
                        .broadcast_to([128, ncur, M])
                    nc.vector.tensor_tensor(
                        ot[:, :ncur, :], dstl_b, iota_b, mybir.AluOpType.is_equal)
                    w_b = w_t[:, c0:c0 + ncur].unsqueeze(2) \
                        .broadcast_to([128, ncur, M])
                    nc.vector.tensor_tensor(
                        ot[:, :ncur, :], ot[:, :ncur, :], w_b,
                        mybir.AluOpType.mult)
                    oh_tiles.append(ot)

                # --- scatter matmuls + per-group PSUM -> SBUF ---
                h_tiles = []
                sum_t = stat_pool.tile([128, p.G], dt.float32, tag="sum")
                sq_t = stat_pool.tile([128, p.G], dt.float32, tag="sq")
                scratch = small_pool.tile([128, BW], dt.float32, tag="scr")
                cur_group = -1
                psum = None

                def close_group(g):
                    hdt = dt.float32 if p.lin else VDT
                    ht = h_pool.tile([128, BW], hdt, tag="ht")
                    nc.scalar.activation(
                        ht[:], psum[:, :],
                        mybir.ActivationFunctionType.Identity,
                        bias=bias_t[:, g:g+1],
                        accum_out=sum_t[:, g:g+1] if (p.lin and not skip_accum) else None,
                    )
                    if p.lin and not skip_accum:
                        nc.scalar.activation(
                            scratch[:], ht[:],
                            mybir.ActivationFunctionType.Square,
                            accum_out=sq_t[:, g:g+1],
                        )
                    h_tiles.append(ht)

                for ci, (g_loc, w_in_g, st, sp) in enumerate(p.sched):
                    if g_loc != cur_group:
                        if cur_group >= 0:
                            close_group(cur_group)
                        cur_group = g_loc
                        psum = ps_pool.tile([128, BW], dt.float32)
                    off = w_in_g * M
                    nc.tensor.matmul(
                        psum[off:off + M, :],
                        lhsT=oh_tiles[ci // OHG][:, ci % OHG, :],
                        rhs=vals_tiles[ci // GCH][:, ci % GCH, :BW],
                        start=st,
                        stop=sp,
                        tile_position=(0, off),
                    )
                close_group(cur_group)
                assert len(h_tiles) == p.G, (li, len(h_tiles), p.G)

                # --- batch-norm stats exchange (lin layers) ---
                if p.lin and not (skip_norm or skip_accum):
                    st_in, st_out = stats_of[li]
                    nc.sync.dma_start(st_in[:, :p.G], sum_t[:])
                    nc.sync.dma_start(st_in[:, p.G:], sq_t[:])
                    if collectives:
                        nc.gpsimd.collective_compute(
                            "AllReduce", mybir.AluOpType.add,
                            replica_groups=AR_GROUPS,
                            ins=[st_in[:]], outs=[st_out[:]],
                        )
                    else:
                        nc.sync.dma_start(st_out[:], st_in[:])
                    gsum = stat_pool.tile([128, p.G], dt.float32, tag="gsum")
                    nc.sync.dma_start(gsum[:], st_out[:, :p.G])
                    gsq = stat_pool.tile([128, p.G], dt.float32, tag="gsq")
                    nc.sync.dma_start(gsq[:], st_out[:, p.G:])

                    # mean = gsum/B ; var = gsq/B - mean^2
                    mean = stat_pool.tile([128, p.G], dt.float32, tag="mean")
                    nc.vector.tensor_scalar_mul(mean[:], gsum[:], 1.0 / B)
                    var = stat_pool.tile([128, p.G], dt.float32, tag="var")
                    nc.vector.tensor_scalar_mul(var[:], gsq[:], 1.0 / B)
                    m2 = stat_pool.tile([128, p.G], dt.float32, tag="m2")
                    nc.vector.tensor_tensor(m2[:], mean[:], mean[:],
                                            mybir.AluOpType.mult)
                    nc.vector.tensor_tensor(var[:], var[:], m2[:],
                                            mybir.AluOpType.subtract)
                    # rstd = 1/sqrt(var+eps)
                    nc.vector.tensor_scalar_add(var[:], var[:], EPS_BN)
                    std = stat_pool.tile([128, p.G], dt.float32, tag="std")
                    nc.scalar.activation(std[:], var[:],
                                         mybir.ActivationFunctionType.Sqrt)
                    rstd = stat_pool.tile([128, p.G], dt.float32, tag="rstd")
                    nc.vector.reciprocal(rstd[:], std[:])

                    stage = li // 2
                    scale = stat_pool.tile([128, p.G], dt.float32, tag="scale")
                    shift = stat_pool.tile([128, p.G], dt.float32, tag="shift")
                    if li < 12:
                        gam = small_pool.tile([128, p.G], dt.float32, tag="gam")
                        nc.sync.dma_start(gam[:], ins[f"gammal_{stage}"][:])
                        bet = small_pool.tile([128, p.G], dt.float32, tag="bet")
                        nc.sync.dma_start(bet[:], ins[f"betal_{stage}"][:])
                        nc.vector.tensor_tensor(scale[:], rstd[:], gam[:],
                                                mybir.AluOpType.mult)
                        nc.vector.tensor_tensor(shift[:], mean[:], scale[:],
                                                mybir.AluOpType.mult)
                        nc.vector.tensor_tensor(shift[:], bet[:], shift[:],
                                                mybir.AluOpType.subtract)
                    else:
                        nc.vector.tensor_copy(scale[:], rstd[:])
                        nc.vector.tensor_tensor(shift[:], mean[:], rstd[:],
                                                mybir.AluOpType.mult)
                        nc.vector.tensor_scalar_mul(shift[:], shift[:], -1.0)

                    # normalize (+relu+mask except final layer)
                    func = (mybir.ActivationFunctionType.Relu if li < 12
                            else mybir.ActivationFunctionType.Identity)
                    for g in range(p.G):
                        ht = h_tiles[g]
                        if li < 12:
                            ht2 = h_pool.tile([128, BW], VDT, tag="ht2")
                            nc.scalar.activation(ht2[:], ht[:], func,
                                                 scale=scale[:, g:g+1],
                                                 bias=shift[:, g:g+1])
                            mt = small_pool.tile([128, BW], VDT, tag="mask")
                            nc.sync.dma_start(
                                mt[:],
                                ins[f"mask_{stage}"][g * GROUP:(g + 1) * GROUP, :])
                            nc.vector.tensor_tensor(ht2[:], ht2[:], mt[:],
                                                    mybir.AluOpType.mult)
                            h_tiles[g] = ht2
                        else:
                            nc.scalar.activation(ht[:], ht[:], func,
                                                 scale=scale[:, g:g+1],
                                                 bias=shift[:, g:g+1])

                # --- write out + allgather next table ---
                if li < limit - 1:
                    for g in range(p.G):
                        nc.sync.dma_start(
                            ag_ins[li][g * GROUP:(g + 1) * GROUP, :BW],
                            h_tiles[g][:])
                    if collectives:
                        nc.gpsimd.collective_compute(
                            "AllGather", mybir.AluOpType.bypass,
                            replica_groups=AG_GROUPS,
                            ins=[ag_ins[li][:]],
                            outs=[tables[li][:]],
                        )
                    else:
                        for qq in range(4):
                            nc.sync.dma_start(
                                tables[li][qq * p.P:(qq + 1) * p.P, :],
                                ag_ins[li][:])
                else:
                    for g in range(p.G):
                        nc.sync.dma_start(
                            out_final[g * GROUP:(g + 1) * GROUP, :],
                            h_tiles[g][:])

    nc.compile()
    return nc


# ---------------- top level -------------------------------------------------
_CACHE = {}
_LAST_RUN = {}
LAST_EXEC_NS = None


def _make_runner(nc, n_cores=N_CORES):
    """jit-compiled dispatcher with device-resident inputs (timing only)."""
    import jax
    from jax.sharding import Mesh, PartitionSpec
    from jax.experimental.shard_map import shard_map
    from concourse import bass2jax
    from concourse.bass2jax import _bass_exec_p, install_neuronx_cc_hook
    install_neuronx_cc_hook()

    partition_name = (nc.partition_id_tensor.name
                      if nc.partition_id_tensor else None)
    in_names, out_names, out_avals, zero_outs = [], [], [], []
    for alloc in nc.m.functions[0].allocations:
        if not isinstance(alloc, mybir.MemoryLocationSet):
            continue
        name = alloc.memorylocations[0].name
        if alloc.kind == "ExternalInput":
            if name != partition_name:
                in_names.append(name)
        elif alloc.kind == "ExternalOutput":
            out_names.append(name)
            out_avals.append(jax.core.ShapedArray(
                tuple(alloc.tensor_shape), mybir.dt.np(alloc.dtype)))
            zero_outs.append(np.zeros(tuple(alloc.tensor_shape),
                                      mybir.dt.np(alloc.dtype)))
    n_params = len(in_names)
    all_in = list(in_names) + list(out_names) + (
        [partition_name] if partition_name else [])

    def _body(*args):
        operands = list(args)
        if partition_name is not None:
            operands.append(bass2jax.partition_id_tensor())
        return tuple(_bass_exec_p.bind(
            *operands, out_avals=tuple(out_avals), in_names=tuple(all_in),
            out_names=tuple(out_names), lowering_input_output_aliases=(),
            sim_require_finite=True, sim_require_nnan=True, nc=nc))

    devices = jax.devices()[:n_cores]
    mesh = Mesh(np.asarray(devices), ("core",))
    nin = n_params + len(out_names)
    donate = tuple(range(n_params, n_params + len(out_names)))
    fn = jax.jit(shard_map(_body, mesh=mesh,
                           in_specs=(PartitionSpec("core"),) * nin,
                           out_specs=(PartitionSpec("core"),) * len(out_names),
                           check_rep=False),
                 donate_argnums=donate, keep_unused=True)
    return fn, in_names, zero_outs


def _dispatch_median(nc, in_maps, reps=8):
    import jax
    import time as _time
    fn, in_names, zero_outs = _make_runner(nc)
    cat = [np.concatenate([np.asarray(in_maps[c][n]) for c in range(N_CORES)], 0)
           for n in in_names]
    zcat = [np.concatenate([z] * N_CORES, 0) for z in zero_outs]
    in_args = [jax.device_put(a) for a in cat]
    out = fn(*in_args, *[jax.device_put(z) for z in zcat])
    jax.block_until_ready(out)
    ts = []
    for _ in range(reps):
        zs = [jax.device_put(z) for z in zcat]
        jax.block_until_ready(zs)
        _time.sleep(0.3)
        t0 = _time.time()
        out = fn(*in_args, *zs)
        jax.block_until_ready(out)
        ts.append(_time.time() - t0)
    return float(np.median(ts))


def _build_null_program(plans):
    """Same external I/O signature as the real program, minimal body."""
    nc = bacc.Bacc("TRN2", target_bir_lowering=False, debug=False,
                   num_devices=N_CORES, dynamic_dma_scratch_size=DMA_SCRATCH)
    nc.dram_tensor("xT", [S_LEVELS[0], BPAD], dt.float32, kind="ExternalInput")
    nc.dram_tensor("iota", [128, M_POOL], dt.float32, kind="ExternalInput")
    for li, p in enumerate(plans):
        nc.dram_tensor(f"idx_{li}", [128, p.NCH * 8], dt.int16, kind="ExternalInput")
        nc.dram_tensor(f"dstl_{li}", [128, p.NCH], dt.float32, kind="ExternalInput")
        nc.dram_tensor(f"w_{li}", [128, p.NCH], dt.float32, kind="ExternalInput")
        nc.dram_tensor(f"biasl_{li}", [128, p.G], dt.float32, kind="ExternalInput")
    for stage in range(6):
        p = plans[2 * stage]
        nc.dram_tensor(f"gammal_{stage}", [128, p.G], dt.float32, kind="ExternalInput")
        nc.dram_tensor(f"betal_{stage}", [128, p.G], dt.float32, kind="ExternalInput")
        nc.dram_tensor(f"mask_{stage}", [p.G * GROUP, BW], dt.float32,
                       kind="ExternalInput")
    pF = plans[-1]
    out_final = nc.dram_tensor("out_final", [pF.G * GROUP, BW], dt.float32,
                               kind="ExternalOutput").ap()
    with tile.TileContext(nc) as tc:
        with tc.tile_pool(name="p", bufs=1) as pool:
            t = pool.tile([128, BW], dt.float32)
            nc.vector.memset(t[:], 0.0)
            for g in range(pF.G):
                nc.sync.dma_start(out_final[g * GROUP:(g + 1) * GROUP, :], t[:])
    nc.compile()
    return nc


def bench_hw_ns():
    """Dispatch-differential estimate of on-device exec time (ns)."""
    if "nc" not in _LAST_RUN:
        return None
    t_real = _dispatch_median(_LAST_RUN["nc"], _LAST_RUN["in_maps"])
    nc_null = _build_null_program(_LAST_RUN["plans"])
    t_null = _dispatch_median(nc_null, _LAST_RUN["in_maps"])
    print(f"[bench] real dispatch median {t_real*1e3:.2f} ms, "
          f"null {t_null*1e3:.2f} ms")
    return max(0.0, (t_real - t_null)) * 1e9


def _host_random():
    """eps per layer and dropout masks via jax CPU (deterministic)."""
    import jax
    cpu = jax.devices("cpu")[0]
    with jax.default_device(cpu):
        noise_key = jax.random.key(1234)
        eps = []
        for li in range(N_LAYERS):
            k = jax.random.fold_in(noise_key, li)
            eps.append(np.asarray(jax.random.normal(k, (NNZ[li],), "float32")))
        masks = []
        for i in range(6):
            k = jax.random.fold_in(noise_key, 100 + i)
            keep = jax.random.bernoulli(k, 1.0 - DROP_P, (B, S_LEVELS[i]))
            masks.append(np.asarray(keep, np.float32) / np.float32(1.0 - DROP_P))
    return eps, masks


def kernel(x, edges, w_mu, w_logsig, bias, gamma, beta):
    x = np.asarray(x, np.float32)
    edges = np.asarray(edges, np.int32)
    w_mu = np.asarray(w_mu, np.float32)
    w_logsig = np.asarray(w_logsig, np.float32)
    bias = np.asarray(bias, np.float32)
    gamma = np.asarray(gamma, np.float32)
    beta = np.asarray(beta, np.float32)

    eps, masks = _host_random()
    w_all = np.concatenate([
        w_mu[NNZ_OFF[li]:NNZ_OFF[li + 1]]
        + np.exp(w_logsig[NNZ_OFF[li]:NNZ_OFF[li + 1]]) * eps[li]
        for li in range(N_LAYERS)
    ]).astype(np.float32)

    plans, data = prepare(edges, w_all, bias)
    _norm_consts_inputs(data, plans, gamma, beta)
    mask_data = _mask_inputs(data, plans, masks)

    key = "prog"
    if key not in _CACHE:
        _CACHE[key] = build_program(plans)
    nc = _CACHE[key]

    iota = np.tile(np.arange(M_POOL, dtype=np.float32), (128, 1))
    in_maps = []
    for core in range(N_CORES):
        q, h = core % 4, core // 4
        m = dict(data[q])
        m.update(mask_data[core])
        xTh = np.zeros((S_LEVELS[0], BPAD), VNP)
        xTh[:, :BW] = x[h * BH:(h + 1) * BH, :].T.astype(VNP)
        m["xT"] = xTh
        m["iota"] = iota
        in_maps.append(m)

    _LAST_RUN.update(nc=nc, in_maps=in_maps, plans=plans)
    res = run_bass_kernel_spmd(nc, in_maps, core_ids=list(range(N_CORES)))

    # ---- assemble full output ----
    pF = plans[-1]
    h_out = np.zeros((B, OUT_SIZE[-1]), np.float32)
    for core in range(N_CORES):
        q, hh = core % 4, core // 4
        nor = pF.node_of_row[q]
        valid = nor >= 0
        block = res.results[core]["out_final"]
        h_out[hh * BH:(hh + 1) * BH, nor[valid]] = block[valid].T

    # ---- KL on host ----
    kl = np.float32(0.0)
    mu64 = w_mu
    ls = w_logsig
    kl_terms = np.exp(2.0 * ls) + mu64 * mu64 - 1.0 - 2.0 * ls
    kl = np.float32(0.5) * np.sum(kl_terms, dtype=np.float32)

    return h_out, np.float32(kl)


# revision 23
# speedup vs baseline: 1.0531x; 1.0531x over previous
"""Trainium2 Bass kernel for nn_BayesianEncoder (gnn_message_passing).

Strategy (8 NeuronCores, one TRN2 chip):
  - Batch (200) split into two halves: cores 0-3 take rows 0:100, cores 4-7
    rows 100:200.  Within each quad the EDGES of every sparse layer are split
    4 ways by (sorted) destination node, so the segment-sum scatter stays
    local per core.
  - Per layer: node-feature table lives in DRAM feature-major (rows = nodes,
    128 f32 cols = padded batch half).  GPSIMD dma_gather pulls one 512B row
    per edge into SBUF as (128 edge-lanes, chunk, 128).  The segment-sum is a
    PE matmul per 128-edge chunk with a weighted one-hot stationary operand
    (built on DVE from an iota compare), accumulating into a PSUM window of
    32 (lin) / 64 (pool) destination rows; 128-row groups of windows share a
    PSUM tile via tile_position column offsets.
  - BatchNorm batch statistics are pair-wise AllReduced across the two batch
    halves ({c, c+4}); normalize+ReLU run on ACT with per-partition
    scale/bias; the dropout mask (host-precomputed, deterministic jax RNG)
    multiplies on DVE.  The normalized slice is AllGathered within each quad
    to rebuild the full gather table for the next layer.
  - Bayesian weights w = mu + exp(logsig)*eps and the KL term are computed on
    host (they only depend on the weight vectors, not on activations).
"""

import os
import numpy as np

import concourse.bacc as bacc
import concourse.tile as tile
from concourse import mybir
from concourse.bass_utils import run_bass_kernel_spmd

dt = mybir.dt

# ---------------- problem constants (hardcoded from the reference) --------
B = 200
S_LEVELS = [30000, 15000, 7500, 3750, 1875, 938, 469]
LIN_DEG, POOL_DEG = 16, 2
EPS_BN = 1e-5
DROP_P = 0.1


def _build_layers():
    layers = [(0, 0, LIN_DEG * S_LEVELS[0])]
    for k in range(6):
        layers.append((k, k + 1, POOL_DEG * S_LEVELS[k]))
        layers.append((k + 1, k + 1, LIN_DEG * S_LEVELS[k + 1]))
    return layers


# ---------------- sharding / kernel config --------------------------------
N_CORES = 8
BPAD = 128         # padded batch columns in gather tables (512B rows)
M_LIN = 32         # one-hot window width, lin layers
M_POOL = 64        # one-hot window width, pool layers
GROUP = 128        # psum group rows (= partitions)
GCH = 8            # chunks per dma_gather instruction
VDT_NAME = os.environ.get("KERNEL_DTYPE", "fp32")
DMA_SCRATCH = 32768   # SBUF SWDGE desc carveout bytes/partition (2048/queue)
VDT = None  # set below once dt import is live
VNP = None


def _set_vdt(name):
    global VDT, VNP, VDT_NAME
    VDT_NAME = name
    VDT = dt.float16 if name == "fp16" else dt.float32
    VNP = np.float16 if name == "fp16" else np.float32
OHG = 16           # chunks per one-hot build instruction


def _set_config(b, s_levels):
    """(Re)compute derived globals.  Exists so a scaled-down config can be
    injected for fast simulator tests; the defaults match the real problem."""
    global B, S_LEVELS, LAYERS, NNZ, OUT_SIZE, IN_SIZE, NNZ_OFF, OUT_OFF
    global BN_OFF, N_LAYERS, IS_LIN, BH, BW
    B = b
    S_LEVELS = s_levels
    LAYERS = _build_layers()
    NNZ = [l[2] for l in LAYERS]
    OUT_SIZE = [S_LEVELS[l[1]] for l in LAYERS]
    IN_SIZE = [S_LEVELS[l[0]] for l in LAYERS]
    NNZ_OFF = np.cumsum([0] + NNZ).tolist()
    OUT_OFF = np.cumsum([0] + OUT_SIZE).tolist()
    BN_OFF = np.cumsum([0] + S_LEVELS[:6]).tolist()
    N_LAYERS = len(LAYERS)
    IS_LIN = [l[0] == l[1] for l in LAYERS]
    BH = B // 2
    BW = BH


_set_config(B, S_LEVELS)
_set_vdt(VDT_NAME)


# ---------------- host-side preprocessing ----------------------------------
def _balanced_group_cuts(edge_group_counts, n_parts):
    """Split groups into n_parts contiguous runs with ~equal edge counts."""
    csum = np.concatenate([[0], np.cumsum(edge_group_counts)])
    total = csum[-1]
    cuts = [0]
    for q in range(1, n_parts):
        target = total * q / n_parts
        g = int(np.searchsorted(csum, target))
        g = max(cuts[-1], min(g, len(edge_group_counts) - (n_parts - q)))
        cuts.append(g)
    cuts.append(len(edge_group_counts))
    return cuts  # group index boundaries, len n_parts+1


def _wrap_idxs(idx_linear):
    """Linear idx list (multiple of 128) -> (128, n/16) int16 wrapped."""
    n = len(idx_linear)
    a = idx_linear.reshape(n // 16, 16).T.astype(np.int16)
    return np.tile(a, (8, 1))


class LayerPlan:
    pass


def prepare(edges, w_all, bias):
    """Plan all 13 layers.  Returns (plans, per_core_data).

    per_core_data: list of 4 dicts (quad position q) with the per-layer input
    arrays.  Cores q and q+4 share everything except the x/mask inputs.
    """
    plans = []
    # per quad position: dict name -> np.ndarray
    data = [dict() for _ in range(4)]

    # node id -> padded table row, for the CURRENT layer input level
    # layer 0 reads x directly: identity mapping
    row_of_node = np.arange(S_LEVELS[0], dtype=np.int64)
    table_rows = S_LEVELS[0]

    for li, (a, b, nnz) in enumerate(LAYERS):
        p = LayerPlan()
        p.li = li
        p.lin = IS_LIN[li]
        p.M = M_LIN if p.lin else M_POOL
        p.win_per_group = GROUP // p.M
        S_out = OUT_SIZE[li]
        e0, e1 = NNZ_OFF[li], NNZ_OFF[li + 1]
        src = np.asarray(edges[0, e0:e1], dtype=np.int64)
        dst = np.asarray(edges[1, e0:e1], dtype=np.int64)
        w = np.asarray(w_all[e0:e1], dtype=np.float32)

        order = np.argsort(dst, kind="stable")
        ssrc, sdst, sw = src[order], dst[order], w[order]

        n_groups_tot = -(-S_out // GROUP)
        grp_of_edge = sdst // GROUP
        grp_counts = np.bincount(grp_of_edge, minlength=n_groups_tot)
        cuts = _balanced_group_cuts(grp_counts, 4)
        p.cuts_groups = cuts
        p.row_start = [c * GROUP for c in cuts[:4]]  # node id where slice starts
        p.G = max(cuts[q + 1] - cuts[q] for q in range(4))  # groups per core
        p.P = p.G * GROUP  # padded rows per core slice

        edge_bounds = np.searchsorted(grp_of_edge, cuts)

        # --- per quad position: window edge counts on the COMMON local grid ---
        n_windows = p.G * p.win_per_group  # local window slots per core
        per_q = []
        win_counts_q = np.zeros((4, n_windows), np.int64)
        for q in range(4):
            lo, hi = edge_bounds[q], edge_bounds[q + 1]
            qsrc, qdst, qw = ssrc[lo:hi], sdst[lo:hi], sw[lo:hi]
            row0 = p.row_start[q]
            win_of_edge = (qdst - row0) // p.M
            win_counts_q[q] = np.bincount(win_of_edge, minlength=n_windows)
            per_q.append((qsrc, qdst, qw, row0))

        # Each core assigns its windows to schedule slots sorted by edge count
        # (descending).  Rank-matching across cores minimizes the union
        # chunk-count padding; the window->slot permutation only relabels
        # output rows, which all downstream consumers read from data tables.
        perm_q = [np.argsort(-win_counts_q[q], kind="stable") for q in range(4)]
        sorted_counts = np.stack(
            [win_counts_q[q][perm_q[q]] for q in range(4)])  # (4, n_windows)
        nck_slot = np.maximum(1, -(-sorted_counts.max(axis=0) // 128))
        sched = []
        for j in range(n_windows):
            g_loc = j // p.win_per_group
            w_in_g = j % p.win_per_group
            n = int(nck_slot[j])
            for ci in range(n):
                sched.append((g_loc, w_in_g, ci == 0, ci == n - 1))
        p.sched = sched
        p.NCH = len(sched)

        # node_of_row[q][r]: node id held at local row r of core q's slice
        p.node_of_row = []
        for q in range(4):
            row0 = per_q[q][3]
            end_q = min(cuts[q + 1] * GROUP, S_out)
            nor = np.full(p.P, -1, np.int64)
            for j in range(n_windows):
                w = int(perm_q[q][j])
                n0 = row0 + w * p.M
                n1 = min(n0 + p.M, end_q)
                if n1 > n0:
                    nor[j * p.M: j * p.M + (n1 - n0)] = np.arange(n0, n1)
            p.node_of_row.append(nor)

        # build lane arrays per core following the union schedule
        for q in range(4):
            qsrc, qdst, qw, row0 = per_q[q]
            woff = np.concatenate([[0], np.cumsum(win_counts_q[q])])
            src_l = np.zeros((p.NCH, 128), np.int64)
            dstl_l = np.full((p.NCH, 128), -1.0, np.float32)
            w_l = np.zeros((p.NCH, 128), np.float32)
            ci = 0
            for j in range(n_windows):
                w = int(perm_q[q][j])
                cnt = int(win_counts_q[q][w])
                base = int(woff[w])
                nbase = row0 + w * p.M  # node id at window start
                for k in range(int(nck_slot[j])):
                    s = base + k * 128
                    ln = max(0, min(128, cnt - k * 128))
                    if ln > 0:
                        src_l[ci, :ln] = row_of_node[qsrc[s:s + ln]]
                        dstl_l[ci, :ln] = (qdst[s:s + ln] - nbase).astype(np.float32)
                        w_l[ci, :ln] = qw[s:s + ln]
                    ci += 1
            assert ci == p.NCH
            data[q][f"idx_{li}"] = _wrap_idxs(src_l.reshape(-1))
            data[q][f"dstl_{li}"] = np.ascontiguousarray(dstl_l.T)  # (128, nch)
            data[q][f"w_{li}"] = np.ascontiguousarray(w_l.T).astype(VNP)  # (128, nch)

            # bias lanes (128, G): lane (p, g) holds bias of node at row g*128+p
            bvec = bias[OUT_OFF[li]:OUT_OFF[li + 1]]
            nor = p.node_of_row[q]
            bl = np.where(nor >= 0, bvec[np.clip(nor, 0, None)], 0.0)
            data[q][f"biasl_{li}"] = np.ascontiguousarray(
                bl.reshape(p.G, GROUP).T.astype(np.float32))

        # mapping node -> padded table row for the NEXT layer input
        row_of_node = np.zeros(S_out, np.int64)
        for q in range(4):
            nor = p.node_of_row[q]
            valid = nor >= 0
            row_of_node[nor[valid]] = q * p.P + np.nonzero(valid)[0]
        table_rows = 4 * p.P
        p.table_rows = table_rows
        assert table_rows < 32768, f"layer {li}: table rows {table_rows} exceed int16"
        plans.append(p)

    return plans, data


def _norm_consts_inputs(data, plans, gamma, beta):
    """gamma/beta lanes per lin stage, laid out like the bias lanes."""
    for stage in range(6):
        li = 2 * stage
        p = plans[li]
        gv = gamma[BN_OFF[stage]:BN_OFF[stage + 1]]
        bv = beta[BN_OFF[stage]:BN_OFF[stage + 1]]
        for q in range(4):
            nor = p.node_of_row[q]
            safe = np.clip(nor, 0, None)
            gl = np.where(nor >= 0, gv[safe], 1.0)
            bl = np.where(nor >= 0, bv[safe], 0.0)
            data[q][f"gammal_{stage}"] = np.ascontiguousarray(
                gl.reshape(p.G, GROUP).T.astype(np.float32))
            data[q][f"betal_{stage}"] = np.ascontiguousarray(
                bl.reshape(p.G, GROUP).T.astype(np.float32))


def _mask_inputs(data, plans, masks):
    """Dropout masks (already scaled by 1/(1-p)), feature-major slices.

    masks[stage]: (B, S_stage) f32.  Stored per (quad q, batch half h) as
    (G*128, BW); cores q and q+4 differ here.
    """
    out = [dict() for _ in range(8)]
    for stage in range(6):
        li = 2 * stage
        p = plans[li]
        m = masks[stage]
        for q in range(4):
            nor = p.node_of_row[q]
            safe = np.clip(nor, 0, None)
            for h in range(2):
                sl = m[h * BH:(h + 1) * BH, safe].T.copy()
                sl[nor < 0] = 0.0
                out[h * 4 + q][f"mask_{stage}"] = np.ascontiguousarray(
                    sl.astype(VNP))
    return out


# ---------------- program builder ------------------------------------------
def build_program(plans, collectives=True, limit=None, skip_norm=False, skip_accum=False):
    nc = bacc.Bacc("TRN2", target_bir_lowering=False, debug=False,
                   num_devices=N_CORES if collectives else 1,
                   dynamic_dma_scratch_size=DMA_SCRATCH)

    AG_GROUPS = [[0, 1, 2, 3], [4, 5, 6, 7]]
    AR_GROUPS = [[0, 4], [1, 5], [2, 6], [3, 7]]

    # ---- dram I/O ----
    xT = nc.dram_tensor("xT", [S_LEVELS[0], BPAD], VDT,
                        kind="ExternalInput").ap()
    iota_in = nc.dram_tensor("iota", [128, M_POOL], dt.float32,
                             kind="ExternalInput").ap()
    ins = {}
    for li, p in enumerate(plans):
        ins[f"idx_{li}"] = nc.dram_tensor(
            f"idx_{li}", [128, p.NCH * 8], dt.int16, kind="ExternalInput").ap()
        ins[f"dstl_{li}"] = nc.dram_tensor(
            f"dstl_{li}", [128, p.NCH], dt.float32, kind="ExternalInput").ap()
        ins[f"w_{li}"] = nc.dram_tensor(
            f"w_{li}", [128, p.NCH], VDT, kind="ExternalInput").ap()
        ins[f"biasl_{li}"] = nc.dram_tensor(
            f"biasl_{li}", [128, p.G], dt.float32, kind="ExternalInput").ap()
    for stage in range(6):
        p = plans[2 * stage]
        ins[f"gammal_{stage}"] = nc.dram_tensor(
            f"gammal_{stage}", [128, p.G], dt.float32, kind="ExternalInput").ap()
        ins[f"betal_{stage}"] = nc.dram_tensor(
            f"betal_{stage}", [128, p.G], dt.float32, kind="ExternalInput").ap()
        ins[f"mask_{stage}"] = nc.dram_tensor(
            f"mask_{stage}", [p.G * GROUP, BW], VDT,
            kind="ExternalInput").ap()

    if limit is None:
        limit = len(plans)
    pF = plans[limit - 1]
    out_final = nc.dram_tensor("out_final", [pF.G * GROUP, BW], dt.float32,
                               kind="ExternalOutput").ap()

    # internal DRAM: gather tables (allgather outputs) + bounce buffers
    tables = []
    ag_ins = []
    for li, p in enumerate(plans[:-1]):
        tables.append(nc.dram_tensor(
            f"table_{li}", [4 * p.P, BPAD], VDT).ap())
        ag_ins.append(nc.dram_tensor(
            f"agin_{li}", [p.P, BPAD], VDT).ap())
    stats_bounce = []
    lin_layers = [li for li in range(N_LAYERS) if IS_LIN[li]]
    for li in lin_layers:
        p = plans[li]
        stats_bounce.append((
            nc.dram_tensor(f"stin_{li}", [128, 2 * p.G], dt.float32).ap(),
            nc.dram_tensor(f"stout_{li}", [128, 2 * p.G], dt.float32).ap(),
        ))
    stats_of = {li: stats_bounce[i] for i, li in enumerate(lin_layers)}

    with tile.TileContext(nc) as tc:
        with (
            tc.tile_pool(name="vals", bufs=5) as vals_pool,
            tc.tile_pool(name="oh", bufs=6) as oh_pool,
            tc.tile_pool(name="meta", bufs=2) as meta_pool,
            tc.tile_pool(name="h", bufs=64) as h_pool,
            tc.tile_pool(name="stat", bufs=4) as stat_pool,
            tc.tile_pool(name="small", bufs=8) as small_pool,
            tc.tile_pool(name="cst", bufs=1) as cst_pool,
            tc.tile_pool(name="ps", bufs=6, space="PSUM") as ps_pool,
        ):
            iota_t = cst_pool.tile([128, M_POOL], dt.float32)
            nc.sync.dma_start(iota_t[:], iota_in[:])

            for li, p in enumerate(plans[:limit]):
                M = p.M
                table = xT if li == 0 else tables[li - 1][:]

                # --- per-layer metadata loads ---
                idx_t = meta_pool.tile([128, p.NCH * 8], dt.int16, tag="idx")
                nc.sync.dma_start(idx_t[:], ins[f"idx_{li}"][:])
                dstl_t = meta_pool.tile([128, p.NCH], dt.float32, tag="dstl")
                nc.sync.dma_start(dstl_t[:], ins[f"dstl_{li}"][:])
                w_t = meta_pool.tile([128, p.NCH], VDT, tag="w")
                nc.sync.dma_start(w_t[:], ins[f"w_{li}"][:])
                bias_t = small_pool.tile([128, p.G], dt.float32, tag="bias")
                nc.sync.dma_start(bias_t[:], ins[f"biasl_{li}"][:])

                # --- gathers ---
                n_g = -(-p.NCH // GCH)
                vals_tiles = []
                for gi in range(n_g):
                    c0 = gi * GCH
                    ncur = min(GCH, p.NCH - c0)
                    vt = vals_pool.tile([128, GCH, BPAD], VDT)
                    nc.gpsimd.dma_gather(
                        out_ap=vt[:, :ncur, :],
                        in_ap=table,
                        idxs_ap=idx_t[:, c0 * 8:(c0 + ncur) * 8],
                        num_idxs=ncur * 128,
                        num_idxs_reg=ncur * 128,
                        elem_size=BPAD,
                    )
                    vals_tiles.append(vt)

                # --- one-hot builds ---
                n_oh = -(-p.NCH // OHG)
                oh_tiles = []
                for oi in range(n_oh):
                    c0 = oi * OHG
                    ncur = min(OHG, p.NCH - c0)
                    ot = oh_pool.tile([128, OHG, M], VDT)
                    dstl_b = dstl_t[:, c0:c0 + ncur].unsqueeze(2) \
                        .broadcast_to([128, ncur, M])
                    iota_b = iota_t[:, :M].unsqueeze(1) \
                        .broadcast_to([128, ncur, M])
                    nc.vector.tensor_tensor(
                        ot[:, :ncur, :], dstl_b, iota_b, mybir.AluOpType.is_equal)
                    w_b = w_t[:, c0:c0 + ncur].unsqueeze(2) \
                        .broadcast_to([128, ncur, M])
                    nc.vector.tensor_tensor(
                        ot[:, :ncur, :], ot[:, :ncur, :], w_b,
                        mybir.AluOpType.mult)
                    oh_tiles.append(ot)

                # --- scatter matmuls + per-group PSUM -> SBUF ---
                h_tiles = []
                sum_t = stat_pool.tile([128, p.G], dt.float32, tag="sum")
                sq_t = stat_pool.tile([128, p.G], dt.float32, tag="sq")
                scratch = small_pool.tile([128, BW], dt.float32, tag="scr")
                cur_group = -1
                psum = None

                def close_group(g):
                    hdt = dt.float32 if p.lin else VDT
                    ht = h_pool.tile([128, BW], hdt, tag="ht")
                    nc.scalar.activation(
                        ht[:], psum[:, :],
                        mybir.ActivationFunctionType.Identity,
                        bias=bias_t[:, g:g+1],
                        accum_out=sum_t[:, g:g+1] if (p.lin and not skip_accum) else None,
                    )
                    if p.lin and not skip_accum:
                        nc.scalar.activation(
                            scratch[:], ht[:],
                            mybir.ActivationFunctionType.Square,
                            accum_out=sq_t[:, g:g+1],
                        )
                    h_tiles.append(ht)

                for ci, (g_loc, w_in_g, st, sp) in enumerate(p.sched):
                    if g_loc != cur_group:
                        if cur_group >= 0:
                            close_group(cur_group)
                        cur_group = g_loc
                        psum = ps_pool.tile([128, BW], dt.float32)
                    off = w_in_g * M
                    nc.tensor.matmul(
                        psum[off:off + M, :],
                        lhsT=oh_tiles[ci // OHG][:, ci % OHG, :],
                        rhs=vals_tiles[ci // GCH][:, ci % GCH, :BW],
                        start=st,
                        stop=sp,
                        tile_position=(0, off),
                    )
                close_group(cur_group)
                assert len(h_tiles) == p.G, (li, len(h_tiles), p.G)

                # --- batch-norm stats exchange (lin layers) ---
                if p.lin and not (skip_norm or skip_accum):
                    st_in, st_out = stats_of[li]
                    nc.sync.dma_start(st_in[:, :p.G], sum_t[:])
                    nc.sync.dma_start(st_in[:, p.G:], sq_t[:])
                    if collectives:
                        nc.gpsimd.collective_compute(
                            "AllReduce", mybir.AluOpType.add,
                            replica_groups=AR_GROUPS,
                            ins=[st_in[:]], outs=[st_out[:]],
                        )
                    else:
                        nc.sync.dma_start(st_out[:], st_in[:])
                    gsum = stat_pool.tile([128, p.G], dt.float32, tag="gsum")
                    nc.sync.dma_start(gsum[:], st_out[:, :p.G])
                    gsq = stat_pool.tile([128, p.G], dt.float32, tag="gsq")
                    nc.sync.dma_start(gsq[:], st_out[:, p.G:])

                    # mean = gsum/B ; var = gsq/B - mean^2
                    mean = stat_pool.tile([128, p.G], dt.float32, tag="mean")
                    nc.vector.tensor_scalar_mul(mean[:], gsum[:], 1.0 / B)
                    var = stat_pool.tile([128, p.G], dt.float32, tag="var")
                    nc.vector.tensor_scalar_mul(var[:], gsq[:], 1.0 / B)
                    m2 = stat_pool.tile([128, p.G], dt.float32, tag="m2")
                    nc.vector.tensor_tensor(m2[:], mean[:], mean[:],
                                            mybir.AluOpType.mult)
                    nc.vector.tensor_tensor(var[:], var[:], m2[:],
                                            mybir.AluOpType.subtract)
                    # rstd = 1/sqrt(var+eps)
                    nc.vector.tensor_scalar_add(var[:], var[:], EPS_BN)
                    std = stat_pool.tile([128, p.G], dt.float32, tag="std")
                    nc.scalar.activation(std[:], var[:],
                                         mybir.ActivationFunctionType.Sqrt)
                    rstd = stat_pool.tile([128, p.G], dt.float32, tag="rstd")
                    nc.vector.reciprocal(rstd[:], std[:])

                    stage = li // 2
                    scale = stat_pool.tile([128, p.G], dt.float32, tag="scale")
                    shift = stat_pool.tile([128, p.G], dt.float32, tag="shift")
                    if li < 12:
                        gam = small_pool.tile([128, p.G], dt.float32, tag="gam")
                        nc.sync.dma_start(gam[:], ins[f"gammal_{stage}"][:])
                        bet = small_pool.tile([128, p.G], dt.float32, tag="bet")
                        nc.sync.dma_start(bet[:], ins[f"betal_{stage}"][:])
                        nc.vector.tensor_tensor(scale[:], rstd[:], gam[:],
                                                mybir.AluOpType.mult)
                        nc.vector.tensor_tensor(shift[:], mean[:], scale[:],
                                                mybir.AluOpType.mult)
                        nc.vector.tensor_tensor(shift[:], bet[:], shift[:],
                                                mybir.AluOpType.subtract)
                    else:
                        nc.vector.tensor_copy(scale[:], rstd[:])
                        nc.vector.tensor_tensor(shift[:], mean[:], rstd[:],
                                                mybir.AluOpType.mult)
                        nc.vector.tensor_scalar_mul(shift[:], shift[:], -1.0)

                    # normalize (+relu+mask except final layer)
                    func = (mybir.ActivationFunctionType.Relu if li < 12
                            else mybir.ActivationFunctionType.Identity)
                    for g in range(p.G):
                        ht = h_tiles[g]
                        if li < 12:
                            ht2 = h_pool.tile([128, BW], VDT, tag="ht2")
                            nc.scalar.activation(ht2[:], ht[:], func,
                                                 scale=scale[:, g:g+1],
                                                 bias=shift[:, g:g+1])
                            mt = small_pool.tile([128, BW], VDT, tag="mask")
                            nc.sync.dma_start(
                                mt[:],
                                ins[f"mask_{stage}"][g * GROUP:(g + 1) * GROUP, :])
                            nc.vector.tensor_tensor(ht2[:], ht2[:], mt[:],
                                                    mybir.AluOpType.mult)
                            h_tiles[g] = ht2
                        else:
                            nc.scalar.activation(ht[:], ht[:], func,
                                                 scale=scale[:, g:g+1],
                                                 bias=shift[:, g:g+1])

                # --- write out + allgather next table ---
                if li < limit - 1:
                    for g in range(p.G):
                        nc.sync.dma_start(
                            ag_ins[li][g * GROUP:(g + 1) * GROUP, :BW],
                            h_tiles[g][:])
                    if collectives:
                        nc.gpsimd.collective_compute(
                            "AllGather", mybir.AluOpType.bypass,
                            replica_groups=AG_GROUPS,
                            ins=[ag_ins[li][:]],
                            outs=[tables[li][:]],
                        )
                    else:
                        for qq in range(4):
                            nc.sync.dma_start(
                                tables[li][qq * p.P:(qq + 1) * p.P, :],
                                ag_ins[li][:])
                else:
                    for g in range(p.G):
                        nc.sync.dma_start(
                            out_final[g * GROUP:(g + 1) * GROUP, :],
                            h_tiles[g][:])

    nc.compile()
    return nc


# ---------------- top level -------------------------------------------------
_CACHE = {}
_LAST_RUN = {}
LAST_EXEC_NS = None


def _make_runner(nc, n_cores=N_CORES):
    """jit-compiled dispatcher with device-resident inputs (timing only)."""
    import jax
    from jax.sharding import Mesh, PartitionSpec
    from jax.experimental.shard_map import shard_map
    from concourse import bass2jax
    from concourse.bass2jax import _bass_exec_p, install_neuronx_cc_hook
    install_neuronx_cc_hook()

    partition_name = (nc.partition_id_tensor.name
                      if nc.partition_id_tensor else None)
    in_names, out_names, out_avals, zero_outs = [], [], [], []
    for alloc in nc.m.functions[0].allocations:
        if not isinstance(alloc, mybir.MemoryLocationSet):
            continue
        name = alloc.memorylocations[0].name
        if alloc.kind == "ExternalInput":
            if name != partition_name:
                in_names.append(name)
        elif alloc.kind == "ExternalOutput":
            out_names.append(name)
            out_avals.append(jax.core.ShapedArray(
                tuple(alloc.tensor_shape), mybir.dt.np(alloc.dtype)))
            zero_outs.append(np.zeros(tuple(alloc.tensor_shape),
                                      mybir.dt.np(alloc.dtype)))
    n_params = len(in_names)
    all_in = list(in_names) + list(out_names) + (
        [partition_name] if partition_name else [])

    def _body(*args):
        operands = list(args)
        if partition_name is not None:
            operands.append(bass2jax.partition_id_tensor())
        return tuple(_bass_exec_p.bind(
            *operands, out_avals=tuple(out_avals), in_names=tuple(all_in),
            out_names=tuple(out_names), lowering_input_output_aliases=(),
            sim_require_finite=True, sim_require_nnan=True, nc=nc))

    devices = jax.devices()[:n_cores]
    mesh = Mesh(np.asarray(devices), ("core",))
    nin = n_params + len(out_names)
    donate = tuple(range(n_params, n_params + len(out_names)))
    fn = jax.jit(shard_map(_body, mesh=mesh,
                           in_specs=(PartitionSpec("core"),) * nin,
                           out_specs=(PartitionSpec("core"),) * len(out_names),
                           check_rep=False),
                 donate_argnums=donate, keep_unused=True)
    return fn, in_names, zero_outs


def _dispatch_median(nc, in_maps, reps=8):
    import jax
    import time as _time
    fn, in_names, zero_outs = _make_runner(nc)
    cat = [np.concatenate([np.asarray(in_maps[c][n]) for c in range(N_CORES)], 0)
           for n in in_names]
    zcat = [np.concatenate([z] * N_CORES, 0) for z in zero_outs]
    in_args = [jax.device_put(a) for a in cat]
    out = fn(*in_args, *[jax.device_put(z) for z in zcat])
    jax.block_until_ready(out)
    ts = []
    for _ in range(reps):
        zs = [jax.device_put(z) for z in zcat]
        jax.block_until_ready(zs)
        _time.sleep(0.3)
        t0 = _time.time()
        out = fn(*in_args, *zs)
        jax.block_until_ready(out)
        ts.append(_time.time() - t0)
    return float(np.median(ts))


def _build_null_program(plans):
    """Same external I/O signature as the real program, minimal body."""
    nc = bacc.Bacc("TRN2", target_bir_lowering=False, debug=False,
                   num_devices=N_CORES, dynamic_dma_scratch_size=DMA_SCRATCH)
    nc.dram_tensor("xT", [S_LEVELS[0], BPAD], dt.float32, kind="ExternalInput")
    nc.dram_tensor("iota", [128, M_POOL], dt.float32, kind="ExternalInput")
    for li, p in enumerate(plans):
        nc.dram_tensor(f"idx_{li}", [128, p.NCH * 8], dt.int16, kind="ExternalInput")
        nc.dram_tensor(f"dstl_{li}", [128, p.NCH], dt.float32, kind="ExternalInput")
        nc.dram_tensor(f"w_{li}", [128, p.NCH], dt.float32, kind="ExternalInput")
        nc.dram_tensor(f"biasl_{li}", [128, p.G], dt.float32, kind="ExternalInput")
    for stage in range(6):
        p = plans[2 * stage]
        nc.dram_tensor(f"gammal_{stage}", [128, p.G], dt.float32, kind="ExternalInput")
        nc.dram_tensor(f"betal_{stage}", [128, p.G], dt.float32, kind="ExternalInput")
        nc.dram_tensor(f"mask_{stage}", [p.G * GROUP, BW], dt.float32,
                       kind="ExternalInput")
    pF = plans[-1]
    out_final = nc.dram_tensor("out_final", [pF.G * GROUP, BW], dt.float32,
                               kind="ExternalOutput").ap()
    with tile.TileContext(nc) as tc:
        with tc.tile_pool(name="p", bufs=1) as pool:
            t = pool.tile([128, BW], dt.float32)
            nc.vector.memset(t[:], 0.0)
            for g in range(pF.G):
                nc.sync.dma_start(out_final[g * GROUP:(g + 1) * GROUP, :], t[:])
    nc.compile()
    return nc


def bench_hw_ns():
    """Dispatch-differential estimate of on-device exec time (ns).

    Opt-in via KERNEL_HW_BENCH=1: re-dispatching the collective NEFF
    through a bare shard_map has been observed to desync the axon mesh."""
    if os.environ.get("KERNEL_HW_BENCH") != "1" or "nc" not in _LAST_RUN:
        return None
    t_real = _dispatch_median(_LAST_RUN["nc"], _LAST_RUN["in_maps"])
    nc_null = _build_null_program(_LAST_RUN["plans"])
    t_null = _dispatch_median(nc_null, _LAST_RUN["in_maps"])
    print(f"[bench] real dispatch median {t_real*1e3:.2f} ms, "
          f"null {t_null*1e3:.2f} ms")
    return max(0.0, (t_real - t_null)) * 1e9


def _host_random():
    """eps per layer and dropout masks via jax CPU (deterministic)."""
    import jax
    cpu = jax.devices("cpu")[0]
    with jax.default_device(cpu):
        noise_key = jax.random.key(1234)
        eps = []
        for li in range(N_LAYERS):
            k = jax.random.fold_in(noise_key, li)
            eps.append(np.asarray(jax.random.normal(k, (NNZ[li],), "float32")))
        masks = []
        for i in range(6):
            k = jax.random.fold_in(noise_key, 100 + i)
            keep = jax.random.bernoulli(k, 1.0 - DROP_P, (B, S_LEVELS[i]))
            masks.append(np.asarray(keep, np.float32) / np.float32(1.0 - DROP_P))
    return eps, masks


def kernel(x, edges, w_mu, w_logsig, bias, gamma, beta):
    x = np.asarray(x, np.float32)
    edges = np.asarray(edges, np.int32)
    w_mu = np.asarray(w_mu, np.float32)
    w_logsig = np.asarray(w_logsig, np.float32)
    bias = np.asarray(bias, np.float32)
    gamma = np.asarray(gamma, np.float32)
    beta = np.asarray(beta, np.float32)

    eps, masks = _host_random()
    w_all = np.concatenate([
        w_mu[NNZ_OFF[li]:NNZ_OFF[li + 1]]
        + np.exp(w_logsig[NNZ_OFF[li]:NNZ_OFF[li + 1]]) * eps[li]
        for li in range(N_LAYERS)
    ]).astype(np.float32)

    plans, data = prepare(edges, w_all, bias)
    _norm_consts_inputs(data, plans, gamma, beta)
    mask_data = _mask_inputs(data, plans, masks)

    key = "prog"
    if key not in _CACHE:
        _CACHE[key] = build_program(plans)
    nc = _CACHE[key]

    iota = np.tile(np.arange(M_POOL, dtype=np.float32), (128, 1))
    in_maps = []
    for core in range(N_CORES):
        q, h = core % 4, core // 4
        m = dict(data[q])
        m.update(mask_data[core])
        xTh = np.zeros((S_LEVELS[0], BPAD), VNP)
        xTh[:, :BW] = x[h * BH:(h + 1) * BH, :].T.astype(VNP)
        m["xT"] = xTh
        m["iota"] = iota
        in_maps.append(m)

    _LAST_RUN.update(nc=nc, in_maps=in_maps, plans=plans)
    res = run_bass_kernel_spmd(nc, in_maps, core_ids=list(range(N_CORES)))

    # ---- assemble full output ----
    pF = plans[-1]
    h_out = np.zeros((B, OUT_SIZE[-1]), np.float32)
    for core in range(N_CORES):
        q, hh = core % 4, core // 4
        nor = pF.node_of_row[q]
        valid = nor >= 0
        block = res.results[core]["out_final"]
        h_out[hh * BH:(hh + 1) * BH, nor[valid]] = block[valid].T

    # ---- KL on host ----
    kl = np.float32(0.0)
    mu64 = w_mu
    ls = w_logsig
    kl_terms = np.exp(2.0 * ls) + mu64 * mu64 - 1.0 - 2.0 * ls
    kl = np.float32(0.5) * np.sum(kl_terms, dtype=np.float32)

    return h_out, np.float32(kl)
